# revision 12
# baseline (speedup 1.0000x reference)
"""DGCNN Bass/Trainium2 kernel.

Strategy (data-parallel over the 8 graphs, one graph per NeuronCore):

Per EdgeConv layer, exploit:
  [x_i, x_j - x_i] @ W  ==  x_i @ (W_hi - W_lo) + x_j @ W_lo
and the monotonicity of LeakyReLU + per-channel affine BN (folded into the
weights on the host) so that
  max_k LReLU(BN(e_ijk))  ==  LReLU(a'_i + max_k b'_{j_k})
with a' = x@U' (+bias via c), b' = x@V' + c.

Neighbor selection maximizes S[i,j] = 2 x_i.x_j - |x_j|^2  (equivalent
ordering to minimizing the squared distance).  S row chunks are computed on
the TensorEngine, top-20 per row extracted with the DVE max8 /
match_replace / max_index instructions (3 rounds of 8), and the b' rows of
the selected neighbors are fetched with an indirect (gather) DMA from a
DRAM staging table, then max-reduced on the DVE.

Everything is fp32.  All feature maps stay on-chip in channel-major
[F, N] layout; per-chunk point-major results are transposed back with the
TensorEngine.
"""

import numpy as np

import concourse.bass as bass
import concourse.bacc as bacc
import concourse.mybir as mybir
import concourse.tile as tile
from concourse.bass import IndirectOffsetOnAxis
from concourse.bass_utils import run_bass_kernel_spmd
from concourse.masks import make_identity

F32 = mybir.dt.float32
U32 = mybir.dt.uint32
AFT = mybir.ActivationFunctionType
ALU = mybir.AluOpType
AXL = mybir.AxisListType

N = 2048
P = 128
NCH = N // P            # 16 row chunks
K = 20
NEG_BIG = -1.0e38
EPS = 1e-5
B = 8
LAYERS = [(3, 64), (64, 64), (64, 128), (128, 256)]


# --------------------------------------------------------------------------
# program builder (single core; one full graph per core)
# --------------------------------------------------------------------------

def _emit_edge_layer(nc, pools, consts, li, din, dout, xT, U_d, V_d, c_d, b_d,
                     out_tiles):
    """xT: AP [din(+1), N] channel-major input; for layers 1-3 the extra last
    row holds ones (contract-augmentation).  out_tiles: SBUF tiles receiving
    the channel-major output (rows beyond dout, if any, must be pre-set)."""
    sb, wpool, pp = pools["work"], pools["wt"], pools["psum"]
    aug = din < 128          # layers 1-3: ones row folded into xT / V / rhs2

    U_sb = wpool.tile([din, dout], F32, tag="U_sb")
    nc.sync.dma_start(out=U_sb[:], in_=U_d[:])
    vrows = din + 1 if aug else din
    V_sb = wpool.tile([vrows, dout], F32, tag="V_sb")
    nc.sync.dma_start(out=V_sb[:], in_=V_d[:])
    if not aug:
        c_sb = wpool.tile([1, dout], F32, tag="c_sb")
        nc.sync.dma_start(out=c_sb[:], in_=c_d[:])

    ones_col = consts["ones_col"]
    ones_row = consts["ones_row"]
    ident = consts["ident"]

    # squared norms -> -|x_j|^2, and the augmented moving tensor
    # rhs2 rows 0..din-1 = 2*x^T, row din = -|x_j|^2 (layers 1-3)
    sqx = sb.tile([din, N], F32, tag="sqx")
    nc.scalar.activation(sqx[:], xT[:din, :], AFT.Square)
    r2rows = din + 1 if aug else din
    rhs2 = sb.tile([r2rows, N], F32, tag="rhs2")
    nc.scalar.activation(rhs2[:din, :], xT[:din, :], AFT.Copy, scale=2.0)
    negsq_t = sb.tile([1, N], F32, tag="negsq")
    negsq = negsq_t[:]
    for c4 in range(4):
        cs = slice(c4 * 512, (c4 + 1) * 512)
        ps_sq = pp.tile([1, 512], F32, tag="ps_s")
        nc.tensor.matmul(out=ps_sq[:], lhsT=ones_col[:din, :], rhs=sqx[:, cs],
                         start=True, stop=True)
        nc.scalar.activation(negsq[:, cs], ps_sq[:], AFT.Copy, scale=-1.0)
    if aug:
        # engine writes must start at partition 0/32/64/96; DMA has no such
        # restriction, so place the -|x|^2 row at partition `din` via DMA
        nc.sync.dma_start(out=rhs2[din:din + 1, :], in_=negsq_t[:])

    # b' staging table first (only needs xT), so gathers can pipeline
    for r in range(NCH):
        rs = slice(r * P, (r + 1) * P)
        ps_b = pp.tile([P, dout], F32, tag="ps_ab")
        if aug:
            nc.tensor.matmul(out=ps_b[:], lhsT=xT[:, rs], rhs=V_sb[:],
                             start=True, stop=True)
        else:
            nc.tensor.matmul(out=ps_b[:], lhsT=xT[:, rs], rhs=V_sb[:],
                             start=True, stop=False)
            nc.tensor.matmul(out=ps_b[:], lhsT=ones_row[:, rs], rhs=c_sb[:],
                             start=False, stop=True)
        b_sb = sb.tile([P, dout], F32, tag="b_sb")
        nc.scalar.activation(b_sb[:], ps_b[:], AFT.Copy)
        nc.sync.dma_start(out=b_d[rs, :], in_=b_sb[:])

    idx = sb.tile([P, NCH * 24], U32, tag="idx")

    # per chunk: S + top-20 + gathers; combine-ops lag 2 chunks behind so the
    # DVE never stalls on the chunk's own gather DMAs.
    LAG = 2
    Gs = [None] * NCH

    def consume(r):
        rs = slice(r * P, (r + 1) * P)
        G = Gs[r]
        m = sb.tile([P, dout], F32, tag="m")
        nc.vector.tensor_reduce(
            out=m[:], in_=G[:].rearrange("p (k c) -> p c k", k=K, c=dout),
            axis=AXL.X, op=ALU.max)
        ps_a = pp.tile([P, dout], F32, tag="ps_ab")
        nc.tensor.matmul(out=ps_a[:], lhsT=xT[:din, rs], rhs=U_sb[:],
                         start=True, stop=True)
        xn = sb.tile([P, dout], F32, tag="xn")
        nc.vector.tensor_add(out=xn[:], in0=ps_a[:], in1=m[:])
        nc.vector.scalar_tensor_tensor(out=xn[:], in0=xn[:], scalar=0.2,
                                       in1=xn[:], op0=ALU.mult, op1=ALU.max)
        for dc in range((dout + P - 1) // P):
            w = min(P, dout - dc * P)
            ps_t = pp.tile([P, P], F32, tag="ps_t")
            nc.tensor.transpose(out=ps_t[:w, :], in_=xn[:, dc * P: dc * P + w],
                                identity=ident[:])
            nc.scalar.activation(out_tiles[dc][:w, rs], ps_t[:w, :], AFT.Copy)

    for r in range(NCH):
        rs = slice(r * P, (r + 1) * P)
        S = sb.tile([P, N], F32, tag="S")
        for c4 in range(4):
            cs = slice(c4 * 512, (c4 + 1) * 512)
            ps_s = pp.tile([P, 512], F32, tag="ps_s")
            if aug:
                nc.tensor.matmul(out=ps_s[:], lhsT=xT[:, rs], rhs=rhs2[:, cs],
                                 start=True, stop=True)
            else:
                nc.tensor.matmul(out=ps_s[:], lhsT=xT[:, rs], rhs=rhs2[:, cs],
                                 start=True, stop=False)
                nc.tensor.matmul(out=ps_s[:], lhsT=ones_row[:, rs],
                                 rhs=negsq[:, cs], start=False, stop=True)
            nc.scalar.activation(S[:, cs], ps_s[:], AFT.Copy)
        ib = r * 24
        for rnd in range(3):
            vals = sb.tile([P, 8], F32, tag="vals")
            nc.vector.max(out=vals[:], in_=S[:])
            nc.vector.max_index(out=idx[:, ib + 8 * rnd: ib + 8 * rnd + 8],
                                in_max=vals[:], in_values=S[:])
            if rnd < 2:
                nc.vector.match_replace(out=S[:], in_to_replace=vals[:],
                                        in_values=S[:], imm_value=NEG_BIG)

        # one indirect DMA per neighbor rank (HW: one descriptor/partition)
        G = pools["g"].tile([P, K * dout], F32, tag="G")
        Gs[r] = G
        for k in range(K):
            nc.gpsimd.indirect_dma_start(
                out=G[:, k * dout:(k + 1) * dout], out_offset=None, in_=b_d[:],
                in_offset=IndirectOffsetOnAxis(ap=idx[:, ib + k: ib + k + 1],
                                               axis=0))
        if r >= LAG:
            consume(r - LAG)
    for r in range(NCH - LAG, NCH):
        consume(r)


def _emit_head(nc, pools, consts, xts, dram):
    """xts: [x1T(64), x2T(64), x3T(128), x4aT(128), x4bT(128)] channel-major."""
    sb, wpool, pp = pools["work"], pools["wt"], pools["psum"]

    cm_sb = wpool.tile([P, 8], F32, tag="cm_sb")
    nc.sync.dma_start(out=cm_sb[:], in_=dram["cm"][:])
    ca_sb = wpool.tile([P, 4], F32, tag="ca_sb")
    nc.sync.dma_start(out=ca_sb[:], in_=dram["ca"][:])
    cb_sb = wpool.tile([P, 2], F32, tag="cb_sb")
    nc.sync.dma_start(out=cb_sb[:], in_=dram["cb"][:])
    bc_sb = wpool.tile([40, 1], F32, tag="bc_sb")
    nc.sync.dma_start(out=bc_sb[:], in_=dram["bc"][:])

    cchunks = [(0, 64, 0), (64, 128, 1), (128, 256, 2), (256, 384, 3),
               (384, 512, 4)]

    maxp = sb.tile([P, 8], F32, tag="maxp")
    sump = sb.tile([P, 8], F32, tag="sump")
    for mi in range(8):
        ms = slice(mi * P, (mi + 1) * P)
        mx4 = sb.tile([P, 4], F32, tag="mx4")
        sm4 = sb.tile([P, 4], F32, tag="sm4")
        for n4 in range(4):
            ns = slice(n4 * 512, (n4 + 1) * 512)
            ps_h = pp.tile([P, 512], F32, tag="ps_h")
            for i, (cs, ce, xi) in enumerate(cchunks):
                wm_sb = wpool.tile([P, P], F32, tag="wm_sb")
                nc.sync.dma_start(out=wm_sb[:ce - cs, :],
                                  in_=dram["Wm"][cs:ce, ms])
                nc.tensor.matmul(out=ps_h[:], lhsT=wm_sb[:ce - cs, :],
                                 rhs=xts[xi][: ce - cs, ns],
                                 start=(i == 0), stop=(i == 4))
            hm = sb.tile([P, 512], F32, tag="hm")
            nc.scalar.activation(hm[:], ps_h[:], AFT.Identity,
                                 bias=cm_sb[:, mi: mi + 1])
            nc.vector.scalar_tensor_tensor(out=hm[:], in0=hm[:], scalar=0.2,
                                           in1=hm[:], op0=ALU.mult, op1=ALU.max,
                                           accum_out=sm4[:, n4: n4 + 1])
            nc.vector.tensor_reduce(out=mx4[:, n4: n4 + 1], in_=hm[:],
                                    axis=AXL.X, op=ALU.max)
        nc.vector.tensor_reduce(out=maxp[:, mi: mi + 1], in_=mx4[:],
                                axis=AXL.X, op=ALU.max)
        nc.vector.tensor_reduce(out=sump[:, mi: mi + 1], in_=sm4[:],
                                axis=AXL.X, op=ALU.add)

    # pooled [2048] = [max(1024); sum(1024)/N(folded)] ;  ha = lrelu(@Wa + ca)
    ha = sb.tile([P, 4], F32, tag="ha")
    for m4 in range(4):
        ms = slice(m4 * P, (m4 + 1) * P)
        ps1 = pp.tile([P, 1], F32, tag="ps_h")
        for c in range(16):
            wa_sb = wpool.tile([P, P], F32, tag="wa_sb")
            nc.sync.dma_start(out=wa_sb[:],
                              in_=dram["Wa"][c * P:(c + 1) * P, ms])
            rhs = maxp[:, c: c + 1] if c < 8 else sump[:, c - 8: c - 7]
            nc.tensor.matmul(out=ps1[:], lhsT=wa_sb[:], rhs=rhs,
                             start=(c == 0), stop=(c == 15))
        nc.scalar.activation(ha[:, m4: m4 + 1], ps1[:], AFT.Identity,
                             bias=ca_sb[:, m4: m4 + 1])
        nc.vector.scalar_tensor_tensor(
            out=ha[:, m4: m4 + 1], in0=ha[:, m4: m4 + 1], scalar=0.2,
            in1=ha[:, m4: m4 + 1], op0=ALU.mult, op1=ALU.max)

    hb = sb.tile([P, 2], F32, tag="hb")
    for m2 in range(2):
        ms = slice(m2 * P, (m2 + 1) * P)
        ps1 = pp.tile([P, 1], F32, tag="ps_h")
        for c in range(4):
            wb_sb = wpool.tile([P, P], F32, tag="wb_sb")
            nc.sync.dma_start(out=wb_sb[:],
                              in_=dram["Wb"][c * P:(c + 1) * P, ms])
            nc.tensor.matmul(out=ps1[:], lhsT=wb_sb[:], rhs=ha[:, c: c + 1],
                             start=(c == 0), stop=(c == 3))
        nc.scalar.activation(hb[:, m2: m2 + 1], ps1[:], AFT.Identity,
                             bias=cb_sb[:, m2: m2 + 1])
        nc.vector.scalar_tensor_tensor(
            out=hb[:, m2: m2 + 1], in0=hb[:, m2: m2 + 1], scalar=0.2,
            in1=hb[:, m2: m2 + 1], op0=ALU.mult, op1=ALU.max)

    ps_o = pp.tile([40, 1], F32, tag="ps_h")
    for c in range(2):
        wc_sb = wpool.tile([P, 40], F32, tag="wc_sb")
        nc.sync.dma_start(out=wc_sb[:], in_=dram["Wc"][c * P:(c + 1) * P, :])
        nc.tensor.matmul(out=ps_o[:], lhsT=wc_sb[:], rhs=hb[:, c: c + 1],
                         start=(c == 0), stop=(c == 1))
    o_sb = sb.tile([40, 1], F32, tag="o_sb")
    nc.scalar.activation(o_sb[:], ps_o[:], AFT.Identity, bias=bc_sb[:])
    nc.sync.dma_start(out=dram["out"][:], in_=o_sb[:])


def build_program():
    nc = bacc.Bacc("TRN2", target_bir_lowering=False, debug=False,
                   enable_asserts=False)

    dram = {"pos": nc.dram_tensor("pos", [N, 3], F32, kind="ExternalInput")}
    for li, (din, dout) in enumerate(LAYERS, start=1):
        vrows = din + 1 if din < 128 else din
        dram[f"U{li}"] = nc.dram_tensor(f"U{li}", [din, dout], F32,
                                        kind="ExternalInput")
        dram[f"V{li}"] = nc.dram_tensor(f"V{li}", [vrows, dout], F32,
                                        kind="ExternalInput")
        if din == 128:
            dram[f"c{li}"] = nc.dram_tensor(f"c{li}", [1, dout], F32,
                                            kind="ExternalInput")
        dram[f"btab{li}"] = nc.dram_tensor(f"btab{li}", [N, dout], F32,
                                           kind="Internal")
    dram["Wm"] = nc.dram_tensor("Wm", [512, 1024], F32, kind="ExternalInput")
    dram["cm"] = nc.dram_tensor("cm", [P, 8], F32, kind="ExternalInput")
    dram["Wa"] = nc.dram_tensor("Wa", [2048, 512], F32, kind="ExternalInput")
    dram["ca"] = nc.dram_tensor("ca", [P, 4], F32, kind="ExternalInput")
    dram["Wb"] = nc.dram_tensor("Wb", [512, 256], F32, kind="ExternalInput")
    dram["cb"] = nc.dram_tensor("cb", [P, 2], F32, kind="ExternalInput")
    dram["Wc"] = nc.dram_tensor("Wc", [256, 40], F32, kind="ExternalInput")
    dram["bc"] = nc.dram_tensor("bc", [40, 1], F32, kind="ExternalInput")
    dram["out"] = nc.dram_tensor("out", [40, 1], F32, kind="ExternalOutput")

    with tile.TileContext(nc) as tc:
        from contextlib import ExitStack
        with ExitStack() as ctx:
            persist = ctx.enter_context(tc.tile_pool(name="persist", bufs=1))
            pools = {
                "work": ctx.enter_context(tc.tile_pool(name="work", bufs=2)),
                "g": ctx.enter_context(tc.tile_pool(name="g", bufs=3)),
                "wt": ctx.enter_context(tc.tile_pool(name="wt", bufs=2)),
            }

            ident = persist.tile([P, P], F32, tag="ident")
            make_identity(nc, ident[:])
            ones_col = persist.tile([P, 1], F32, tag="ones_col")
            nc.vector.memset(ones_col[:], 1.0)
            ones_row = persist.tile([1, N], F32, tag="ones_row")
            nc.vector.memset(ones_row[:], 1.0)
            consts = {"ident": ident, "ones_col": ones_col,
                      "ones_row": ones_row}

            # x feature tiles (channel-major), kept for the head concat
            x1 = persist.tile([65, N], F32, tag="x1")
            x2 = persist.tile([65, N], F32, tag="x2")
            x3 = persist.tile([P, N], F32, tag="x3")
            x4a = persist.tile([P, N], F32, tag="x4a")
            x4b = persist.tile([P, N], F32, tag="x4b")
            x0 = persist.tile([4, N], F32, tag="x0")

            nc.vector.memset(x0[:], 1.0)
            nc.sync.dma_start(out=x0[:3, :],
                              in_=dram["pos"][:].rearrange("n f -> f n"))
            nc.vector.memset(x1[64:65, :], 1.0)
            nc.vector.memset(x2[64:65, :], 1.0)

            outs = [[x1], [x2], [x3], [x4a, x4b]]
            xin = x0[:]
            with tc.tile_pool(name="psum_l", bufs=2, space="PSUM") as pp_l:
                pools["psum"] = pp_l
                for li, (din, dout) in enumerate(LAYERS, start=1):
                    _emit_edge_layer(nc, pools, consts, li, din, dout, xin,
                                     dram[f"U{li}"], dram[f"V{li}"],
                                     dram.get(f"c{li}"), dram[f"btab{li}"],
                                     outs[li - 1])
                    if li < 4:
                        xin = outs[li - 1][0][:]

            with tc.tile_pool(name="psum_h", bufs=2, space="PSUM") as pp_h:
                pools["psum"] = pp_h
                _emit_head(nc, pools, consts,
                           [x1[:], x2[:], x3[:], x4a[:], x4b[:]], dram)

    nc.compile()
    return nc


# --------------------------------------------------------------------------
# host-side weight folding
# --------------------------------------------------------------------------

def fold_inputs(inputs):
    d = {k: np.ascontiguousarray(np.asarray(v, dtype=np.float32))
         for k, v in inputs.items()}

    def bn(name):
        s = d["g" + name] / np.sqrt(d["var" + name] + np.float32(EPS))
        c = d["beta" + name] - d["mu" + name] * s
        return s.astype(np.float32), c.astype(np.float32)

    common = {}
    for li, (din, dout) in enumerate(LAYERS, start=1):
        W = d[f"W{li}"]
        s, c = bn(str(li))
        Ws = (W * s[None, :]).astype(np.float32)
        common[f"U{li}"] = np.ascontiguousarray(Ws[:din] - Ws[din:])
        V = Ws[din:]
        if din < 128:
            V = np.concatenate([V, c[None, :]], axis=0)
            common[f"V{li}"] = np.ascontiguousarray(V)
        else:
            common[f"V{li}"] = np.ascontiguousarray(V)
            common[f"c{li}"] = np.ascontiguousarray(c[None, :])

    s, c = bn("m")
    common["Wm"] = np.ascontiguousarray(d["Wm"] * s[None, :])
    common["cm"] = np.ascontiguousarray(c.reshape(8, P).T)

    s, c = bn("a")
    Wa = (d["Wa"] * s[None, :]).astype(np.float32)
    Wa[1024:] /= np.float32(N)
    common["Wa"] = np.ascontiguousarray(Wa)
    common["ca"] = np.ascontiguousarray(c.reshape(4, P).T)

    s, _ = bn("b")
    common["Wb"] = np.ascontiguousarray(d["Wb"] * s[None, :])
    cb = ((d["bias_b"] - d["mub"]) * s + d["betab"]).astype(np.float32)
    common["cb"] = np.ascontiguousarray(cb.reshape(2, P).T)

    common["Wc"] = np.ascontiguousarray(d["Wc"])
    common["bc"] = np.ascontiguousarray(d["bias_c"].reshape(40, 1))

    in_maps = [{**common, "pos": np.ascontiguousarray(d["pos"][g])}
               for g in range(B)]
    return in_maps


_CACHE = {}


def kernel(**inputs):
    if "nc" not in _CACHE:
        _CACHE["nc"] = build_program()
    nc = _CACHE["nc"]
    in_maps = fold_inputs(inputs)
    res = run_bass_kernel_spmd(nc, in_maps, core_ids=list(range(B)))
    out = np.stack([np.asarray(r["out"]).reshape(40) for r in res.results])
    return out.astype(np.float32)


# revision 13
# speedup vs baseline: 1.0394x; 1.0394x over previous
"""DGCNN Bass/Trainium2 kernel.

Strategy (data-parallel over the 8 graphs, one graph per NeuronCore):

Per EdgeConv layer, exploit:
  [x_i, x_j - x_i] @ W  ==  x_i @ (W_hi - W_lo) + x_j @ W_lo
and the monotonicity of LeakyReLU + per-channel affine BN (folded into the
weights on the host) so that
  max_k LReLU(BN(e_ijk))  ==  LReLU(a'_i + max_k b'_{j_k})
with a' = x@U' (+bias via c), b' = x@V' + c.

Neighbor selection maximizes S[i,j] = 2 x_i.x_j - |x_j|^2  (equivalent
ordering to minimizing the squared distance).  S row chunks are computed on
the TensorEngine, top-20 per row extracted with the DVE max8 /
match_replace / max_index instructions (3 rounds of 8), and the b' rows of
the selected neighbors are fetched with an indirect (gather) DMA from a
DRAM staging table, then max-reduced on the DVE.

Everything is fp32.  All feature maps stay on-chip in channel-major
[F, N] layout; per-chunk point-major results are transposed back with the
TensorEngine.
"""

import numpy as np

import concourse.bass as bass
import concourse.bacc as bacc
import concourse.mybir as mybir
import concourse.tile as tile
from concourse.bass import IndirectOffsetOnAxis
from concourse.bass_utils import run_bass_kernel_spmd
from concourse.masks import make_identity

F32 = mybir.dt.float32
U32 = mybir.dt.uint32
AFT = mybir.ActivationFunctionType
ALU = mybir.AluOpType
AXL = mybir.AxisListType

N = 2048
P = 128
NCH = N // P            # 16 row chunks
K = 20
NEG_BIG = -1.0e38
EPS = 1e-5
B = 8
LAYERS = [(3, 64), (64, 64), (64, 128), (128, 256)]


# --------------------------------------------------------------------------
# program builder (single core; one full graph per core)
# --------------------------------------------------------------------------

def _emit_edge_layer(nc, pools, consts, li, din, dout, xT, U_d, V_d, c_d, b_d,
                     out_tiles):
    """xT: AP [din(+1), N] channel-major input; for layers 1-3 the extra last
    row holds ones (contract-augmentation).  out_tiles: SBUF tiles receiving
    the channel-major output (rows beyond dout, if any, must be pre-set)."""
    sb, wpool, pp = pools["work"], pools["wt"], pools["psum"]
    aug = din < 128          # layers 1-3: ones row folded into xT / V / rhs2

    U_sb = wpool.tile([din, dout], F32, tag="U_sb")
    nc.sync.dma_start(out=U_sb[:], in_=U_d[:])
    vrows = din + 1 if aug else din
    V_sb = wpool.tile([vrows, dout], F32, tag="V_sb")
    nc.sync.dma_start(out=V_sb[:], in_=V_d[:])
    if not aug:
        c_sb = wpool.tile([1, dout], F32, tag="c_sb")
        nc.sync.dma_start(out=c_sb[:], in_=c_d[:])

    ones_col = consts["ones_col"]
    ones_row = consts["ones_row"]
    ident = consts["ident"]

    # squared norms -> -|x_j|^2, and the augmented moving tensor
    # rhs2 rows 0..din-1 = 2*x^T, row din = -|x_j|^2 (layers 1-3)
    sqx = sb.tile([din, N], F32, tag="sqx")
    nc.scalar.activation(sqx[:], xT[:din, :], AFT.Square)
    r2rows = din + 1 if aug else din
    rhs2 = sb.tile([r2rows, N], F32, tag="rhs2")
    nc.scalar.activation(rhs2[:din, :], xT[:din, :], AFT.Copy, scale=2.0)
    negsq_t = sb.tile([1, N], F32, tag="negsq")
    negsq = negsq_t[:]
    for c4 in range(4):
        cs = slice(c4 * 512, (c4 + 1) * 512)
        ps_sq = pp.tile([1, 512], F32, tag="ps_s")
        nc.tensor.matmul(out=ps_sq[:], lhsT=ones_col[:din, :], rhs=sqx[:, cs],
                         start=True, stop=True)
        nc.scalar.activation(negsq[:, cs], ps_sq[:], AFT.Copy, scale=-1.0)
    if aug:
        # engine writes must start at partition 0/32/64/96; DMA has no such
        # restriction, so place the -|x|^2 row at partition `din` via DMA
        nc.sync.dma_start(out=rhs2[din:din + 1, :], in_=negsq_t[:])

    # b' staging table first (only needs xT), so gathers can pipeline
    for r in range(NCH):
        rs = slice(r * P, (r + 1) * P)
        ps_b = pp.tile([P, dout], F32, tag="ps_ab")
        if aug:
            nc.tensor.matmul(out=ps_b[:], lhsT=xT[:, rs], rhs=V_sb[:],
                             start=True, stop=True)
        else:
            nc.tensor.matmul(out=ps_b[:], lhsT=xT[:, rs], rhs=V_sb[:],
                             start=True, stop=False)
            nc.tensor.matmul(out=ps_b[:], lhsT=ones_row[:, rs], rhs=c_sb[:],
                             start=False, stop=True)
        b_sb = sb.tile([P, dout], F32, tag="b_sb")
        nc.scalar.activation(b_sb[:], ps_b[:], AFT.Copy)
        nc.sync.dma_start(out=b_d[rs, :], in_=b_sb[:])

    idx = sb.tile([P, NCH * 24], U32, tag="idx")

    # per chunk: S + top-20 + gathers; combine-ops lag 2 chunks behind so the
    # DVE never stalls on the chunk's own gather DMAs.
    LAG = 0
    Gs = [None] * NCH

    def consume(r):
        rs = slice(r * P, (r + 1) * P)
        G = Gs[r]
        m = sb.tile([P, dout], F32, tag="m")
        nc.vector.tensor_reduce(
            out=m[:], in_=G[:].rearrange("p (k c) -> p c k", k=K, c=dout),
            axis=AXL.X, op=ALU.max)
        ps_a = pp.tile([P, dout], F32, tag="ps_ab")
        nc.tensor.matmul(out=ps_a[:], lhsT=xT[:din, rs], rhs=U_sb[:],
                         start=True, stop=True)
        xn = sb.tile([P, dout], F32, tag="xn")
        nc.vector.tensor_add(out=xn[:], in0=ps_a[:], in1=m[:])
        nc.vector.scalar_tensor_tensor(out=xn[:], in0=xn[:], scalar=0.2,
                                       in1=xn[:], op0=ALU.mult, op1=ALU.max)
        for dc in range((dout + P - 1) // P):
            w = min(P, dout - dc * P)
            ps_t = pp.tile([P, P], F32, tag="ps_t")
            nc.tensor.transpose(out=ps_t[:w, :], in_=xn[:, dc * P: dc * P + w],
                                identity=ident[:])
            nc.scalar.activation(out_tiles[dc][:w, rs], ps_t[:w, :], AFT.Copy)

    for r in range(NCH):
        rs = slice(r * P, (r + 1) * P)
        S = sb.tile([P, N], F32, tag="S")
        for c4 in range(4):
            cs = slice(c4 * 512, (c4 + 1) * 512)
            ps_s = pp.tile([P, 512], F32, tag="ps_s")
            if aug:
                nc.tensor.matmul(out=ps_s[:], lhsT=xT[:, rs], rhs=rhs2[:, cs],
                                 start=True, stop=True)
            else:
                nc.tensor.matmul(out=ps_s[:], lhsT=xT[:, rs], rhs=rhs2[:, cs],
                                 start=True, stop=False)
                nc.tensor.matmul(out=ps_s[:], lhsT=ones_row[:, rs],
                                 rhs=negsq[:, cs], start=False, stop=True)
            nc.scalar.activation(S[:, cs], ps_s[:], AFT.Copy)
        ib = r * 24
        for rnd in range(3):
            vals = sb.tile([P, 8], F32, tag="vals")
            nc.vector.max(out=vals[:], in_=S[:])
            nc.vector.max_index(out=idx[:, ib + 8 * rnd: ib + 8 * rnd + 8],
                                in_max=vals[:], in_values=S[:])
            if rnd < 2:
                nc.vector.match_replace(out=S[:], in_to_replace=vals[:],
                                        in_values=S[:], imm_value=NEG_BIG)

        # one indirect DMA per neighbor rank (HW: one descriptor/partition)
        G = pools["g"].tile([P, K * dout], F32, tag="G")
        Gs[r] = G
        for k in range(K):
            nc.gpsimd.indirect_dma_start(
                out=G[:, k * dout:(k + 1) * dout], out_offset=None, in_=b_d[:],
                in_offset=IndirectOffsetOnAxis(ap=idx[:, ib + k: ib + k + 1],
                                               axis=0))
        if r >= LAG:
            consume(r - LAG)
    for r in range(NCH - LAG, NCH):
        consume(r)


def _emit_head(nc, pools, consts, xts, dram):
    """xts: [x1T(64), x2T(64), x3T(128), x4aT(128), x4bT(128)] channel-major."""
    sb, wpool, pp = pools["work"], pools["wt"], pools["psum"]

    cm_sb = wpool.tile([P, 8], F32, tag="cm_sb")
    nc.sync.dma_start(out=cm_sb[:], in_=dram["cm"][:])
    ca_sb = wpool.tile([P, 4], F32, tag="ca_sb")
    nc.sync.dma_start(out=ca_sb[:], in_=dram["ca"][:])
    cb_sb = wpool.tile([P, 2], F32, tag="cb_sb")
    nc.sync.dma_start(out=cb_sb[:], in_=dram["cb"][:])
    bc_sb = wpool.tile([40, 1], F32, tag="bc_sb")
    nc.sync.dma_start(out=bc_sb[:], in_=dram["bc"][:])

    cchunks = [(0, 64, 0), (64, 128, 1), (128, 256, 2), (256, 384, 3),
               (384, 512, 4)]

    maxp = sb.tile([P, 8], F32, tag="maxp")
    sump = sb.tile([P, 8], F32, tag="sump")
    for mi in range(8):
        ms = slice(mi * P, (mi + 1) * P)
        mx4 = sb.tile([P, 4], F32, tag="mx4")
        sm4 = sb.tile([P, 4], F32, tag="sm4")
        for n4 in range(4):
            ns = slice(n4 * 512, (n4 + 1) * 512)
            ps_h = pp.tile([P, 512], F32, tag="ps_h")
            for i, (cs, ce, xi) in enumerate(cchunks):
                wm_sb = wpool.tile([P, P], F32, tag="wm_sb")
                nc.sync.dma_start(out=wm_sb[:ce - cs, :],
                                  in_=dram["Wm"][cs:ce, ms])
                nc.tensor.matmul(out=ps_h[:], lhsT=wm_sb[:ce - cs, :],
                                 rhs=xts[xi][: ce - cs, ns],
                                 start=(i == 0), stop=(i == 4))
            hm = sb.tile([P, 512], F32, tag="hm")
            nc.scalar.activation(hm[:], ps_h[:], AFT.Identity,
                                 bias=cm_sb[:, mi: mi + 1])
            nc.vector.scalar_tensor_tensor(out=hm[:], in0=hm[:], scalar=0.2,
                                           in1=hm[:], op0=ALU.mult, op1=ALU.max,
                                           accum_out=sm4[:, n4: n4 + 1])
            nc.vector.tensor_reduce(out=mx4[:, n4: n4 + 1], in_=hm[:],
                                    axis=AXL.X, op=ALU.max)
        nc.vector.tensor_reduce(out=maxp[:, mi: mi + 1], in_=mx4[:],
                                axis=AXL.X, op=ALU.max)
        nc.vector.tensor_reduce(out=sump[:, mi: mi + 1], in_=sm4[:],
                                axis=AXL.X, op=ALU.add)

    # pooled [2048] = [max(1024); sum(1024)/N(folded)] ;  ha = lrelu(@Wa + ca)
    ha = sb.tile([P, 4], F32, tag="ha")
    for m4 in range(4):
        ms = slice(m4 * P, (m4 + 1) * P)
        ps1 = pp.tile([P, 1], F32, tag="ps_h")
        for c in range(16):
            wa_sb = wpool.tile([P, P], F32, tag="wa_sb")
            nc.sync.dma_start(out=wa_sb[:],
                              in_=dram["Wa"][c * P:(c + 1) * P, ms])
            rhs = maxp[:, c: c + 1] if c < 8 else sump[:, c - 8: c - 7]
            nc.tensor.matmul(out=ps1[:], lhsT=wa_sb[:], rhs=rhs,
                             start=(c == 0), stop=(c == 15))
        nc.scalar.activation(ha[:, m4: m4 + 1], ps1[:], AFT.Identity,
                             bias=ca_sb[:, m4: m4 + 1])
        nc.vector.scalar_tensor_tensor(
            out=ha[:, m4: m4 + 1], in0=ha[:, m4: m4 + 1], scalar=0.2,
            in1=ha[:, m4: m4 + 1], op0=ALU.mult, op1=ALU.max)

    hb = sb.tile([P, 2], F32, tag="hb")
    for m2 in range(2):
        ms = slice(m2 * P, (m2 + 1) * P)
        ps1 = pp.tile([P, 1], F32, tag="ps_h")
        for c in range(4):
            wb_sb = wpool.tile([P, P], F32, tag="wb_sb")
            nc.sync.dma_start(out=wb_sb[:],
                              in_=dram["Wb"][c * P:(c + 1) * P, ms])
            nc.tensor.matmul(out=ps1[:], lhsT=wb_sb[:], rhs=ha[:, c: c + 1],
                             start=(c == 0), stop=(c == 3))
        nc.scalar.activation(hb[:, m2: m2 + 1], ps1[:], AFT.Identity,
                             bias=cb_sb[:, m2: m2 + 1])
        nc.vector.scalar_tensor_tensor(
            out=hb[:, m2: m2 + 1], in0=hb[:, m2: m2 + 1], scalar=0.2,
            in1=hb[:, m2: m2 + 1], op0=ALU.mult, op1=ALU.max)

    ps_o = pp.tile([40, 1], F32, tag="ps_h")
    for c in range(2):
        wc_sb = wpool.tile([P, 40], F32, tag="wc_sb")
        nc.sync.dma_start(out=wc_sb[:], in_=dram["Wc"][c * P:(c + 1) * P, :])
        nc.tensor.matmul(out=ps_o[:], lhsT=wc_sb[:], rhs=hb[:, c: c + 1],
                         start=(c == 0), stop=(c == 1))
    o_sb = sb.tile([40, 1], F32, tag="o_sb")
    nc.scalar.activation(o_sb[:], ps_o[:], AFT.Identity, bias=bc_sb[:])
    nc.sync.dma_start(out=dram["out"][:], in_=o_sb[:])


def build_program():
    nc = bacc.Bacc("TRN2", target_bir_lowering=False, debug=False,
                   enable_asserts=False)

    dram = {"pos": nc.dram_tensor("pos", [N, 3], F32, kind="ExternalInput")}
    for li, (din, dout) in enumerate(LAYERS, start=1):
        vrows = din + 1 if din < 128 else din
        dram[f"U{li}"] = nc.dram_tensor(f"U{li}", [din, dout], F32,
                                        kind="ExternalInput")
        dram[f"V{li}"] = nc.dram_tensor(f"V{li}", [vrows, dout], F32,
                                        kind="ExternalInput")
        if din == 128:
            dram[f"c{li}"] = nc.dram_tensor(f"c{li}", [1, dout], F32,
                                            kind="ExternalInput")
        dram[f"btab{li}"] = nc.dram_tensor(f"btab{li}", [N, dout], F32,
                                           kind="Internal")
    dram["Wm"] = nc.dram_tensor("Wm", [512, 1024], F32, kind="ExternalInput")
    dram["cm"] = nc.dram_tensor("cm", [P, 8], F32, kind="ExternalInput")
    dram["Wa"] = nc.dram_tensor("Wa", [2048, 512], F32, kind="ExternalInput")
    dram["ca"] = nc.dram_tensor("ca", [P, 4], F32, kind="ExternalInput")
    dram["Wb"] = nc.dram_tensor("Wb", [512, 256], F32, kind="ExternalInput")
    dram["cb"] = nc.dram_tensor("cb", [P, 2], F32, kind="ExternalInput")
    dram["Wc"] = nc.dram_tensor("Wc", [256, 40], F32, kind="ExternalInput")
    dram["bc"] = nc.dram_tensor("bc", [40, 1], F32, kind="ExternalInput")
    dram["out"] = nc.dram_tensor("out", [40, 1], F32, kind="ExternalOutput")

    with tile.TileContext(nc) as tc:
        from contextlib import ExitStack
        with ExitStack() as ctx:
            persist = ctx.enter_context(tc.tile_pool(name="persist", bufs=1))
            pools = {
                "work": ctx.enter_context(tc.tile_pool(name="work", bufs=2)),
                "g": ctx.enter_context(tc.tile_pool(name="g", bufs=3)),
                "wt": ctx.enter_context(tc.tile_pool(name="wt", bufs=2)),
            }

            ident = persist.tile([P, P], F32, tag="ident")
            make_identity(nc, ident[:])
            ones_col = persist.tile([P, 1], F32, tag="ones_col")
            nc.vector.memset(ones_col[:], 1.0)
            ones_row = persist.tile([1, N], F32, tag="ones_row")
            nc.vector.memset(ones_row[:], 1.0)
            consts = {"ident": ident, "ones_col": ones_col,
                      "ones_row": ones_row}

            # x feature tiles (channel-major), kept for the head concat
            x1 = persist.tile([65, N], F32, tag="x1")
            x2 = persist.tile([65, N], F32, tag="x2")
            x3 = persist.tile([P, N], F32, tag="x3")
            x4a = persist.tile([P, N], F32, tag="x4a")
            x4b = persist.tile([P, N], F32, tag="x4b")
            x0 = persist.tile([4, N], F32, tag="x0")

            nc.vector.memset(x0[:], 1.0)
            nc.sync.dma_start(out=x0[:3, :],
                              in_=dram["pos"][:].rearrange("n f -> f n"))
            nc.vector.memset(x1[64:65, :], 1.0)
            nc.vector.memset(x2[64:65, :], 1.0)

            outs = [[x1], [x2], [x3], [x4a, x4b]]
            xin = x0[:]
            with tc.tile_pool(name="psum_l", bufs=2, space="PSUM") as pp_l:
                pools["psum"] = pp_l
                for li, (din, dout) in enumerate(LAYERS, start=1):
                    _emit_edge_layer(nc, pools, consts, li, din, dout, xin,
                                     dram[f"U{li}"], dram[f"V{li}"],
                                     dram.get(f"c{li}"), dram[f"btab{li}"],
                                     outs[li - 1])
                    if li < 4:
                        xin = outs[li - 1][0][:]

            with tc.tile_pool(name="psum_h", bufs=2, space="PSUM") as pp_h:
                pools["psum"] = pp_h
                _emit_head(nc, pools, consts,
                           [x1[:], x2[:], x3[:], x4a[:], x4b[:]], dram)

    nc.compile()
    return nc


# --------------------------------------------------------------------------
# host-side weight folding
# --------------------------------------------------------------------------

def fold_inputs(inputs):
    d = {k: np.ascontiguousarray(np.asarray(v, dtype=np.float32))
         for k, v in inputs.items()}

    def bn(name):
        s = d["g" + name] / np.sqrt(d["var" + name] + np.float32(EPS))
        c = d["beta" + name] - d["mu" + name] * s
        return s.astype(np.float32), c.astype(np.float32)

    common = {}
    for li, (din, dout) in enumerate(LAYERS, start=1):
        W = d[f"W{li}"]
        s, c = bn(str(li))
        Ws = (W * s[None, :]).astype(np.float32)
        common[f"U{li}"] = np.ascontiguousarray(Ws[:din] - Ws[din:])
        V = Ws[din:]
        if din < 128:
            V = np.concatenate([V, c[None, :]], axis=0)
            common[f"V{li}"] = np.ascontiguousarray(V)
        else:
            common[f"V{li}"] = np.ascontiguousarray(V)
            common[f"c{li}"] = np.ascontiguousarray(c[None, :])

    s, c = bn("m")
    common["Wm"] = np.ascontiguousarray(d["Wm"] * s[None, :])
    common["cm"] = np.ascontiguousarray(c.reshape(8, P).T)

    s, c = bn("a")
    Wa = (d["Wa"] * s[None, :]).astype(np.float32)
    Wa[1024:] /= np.float32(N)
    common["Wa"] = np.ascontiguousarray(Wa)
    common["ca"] = np.ascontiguousarray(c.reshape(4, P).T)

    s, _ = bn("b")
    common["Wb"] = np.ascontiguousarray(d["Wb"] * s[None, :])
    cb = ((d["bias_b"] - d["mub"]) * s + d["betab"]).astype(np.float32)
    common["cb"] = np.ascontiguousarray(cb.reshape(2, P).T)

    common["Wc"] = np.ascontiguousarray(d["Wc"])
    common["bc"] = np.ascontiguousarray(d["bias_c"].reshape(40, 1))

    in_maps = [{**common, "pos": np.ascontiguousarray(d["pos"][g])}
               for g in range(B)]
    return in_maps


_CACHE = {}


def kernel(**inputs):
    if "nc" not in _CACHE:
        _CACHE["nc"] = build_program()
    nc = _CACHE["nc"]
    in_maps = fold_inputs(inputs)
    res = run_bass_kernel_spmd(nc, in_maps, core_ids=list(range(B)))
    out = np.stack([np.asarray(r["out"]).reshape(40) for r in res.results])
    return out.astype(np.float32)


# revision 14
# speedup vs baseline: 1.1192x; 1.0767x over previous
"""DGCNN Bass/Trainium2 kernel.

Strategy (data-parallel over the 8 graphs, one graph per NeuronCore):

Per EdgeConv layer, exploit:
  [x_i, x_j - x_i] @ W  ==  x_i @ (W_hi - W_lo) + x_j @ W_lo
and the monotonicity of LeakyReLU + per-channel affine BN (folded into the
weights on the host) so that
  max_k LReLU(BN(e_ijk))  ==  LReLU(a'_i + max_k b'_{j_k})
with a' = x@U' (+bias via c), b' = x@V' + c.

Neighbor selection maximizes S[i,j] = 2 x_i.x_j - |x_j|^2  (equivalent
ordering to minimizing the squared distance).  S row chunks are computed on
the TensorEngine, top-20 per row extracted with the DVE max8 /
match_replace / max_index instructions (3 rounds of 8), and the b' rows of
the selected neighbors are fetched with an indirect (gather) DMA from a
DRAM staging table, then max-reduced on the DVE.

Everything is fp32.  All feature maps stay on-chip in channel-major
[F, N] layout; per-chunk point-major results are transposed back with the
TensorEngine.
"""

import numpy as np

import concourse.bass as bass
import concourse.bacc as bacc
import concourse.mybir as mybir
import concourse.tile as tile
from concourse.bass import IndirectOffsetOnAxis
from concourse.bass_utils import run_bass_kernel_spmd
from concourse.masks import make_identity

F32 = mybir.dt.float32
U32 = mybir.dt.uint32
AFT = mybir.ActivationFunctionType
ALU = mybir.AluOpType
AXL = mybir.AxisListType

N = 2048
P = 128
NCH = N // P            # 16 row chunks
K = 20
NEG_BIG = -1.0e38
EPS = 1e-5
B = 8
LAYERS = [(3, 64), (64, 64), (64, 128), (128, 256)]


# --------------------------------------------------------------------------
# program builder (single core; one full graph per core)
# --------------------------------------------------------------------------

def _emit_edge_layer(nc, pools, consts, li, din, dout, xT, U_d, V_d, c_d, b_d,
                     out_tiles):
    """xT: AP [din(+1), N] channel-major input; for layers 1-3 the extra last
    row holds ones (contract-augmentation).  out_tiles: SBUF tiles receiving
    the channel-major output (rows beyond dout, if any, must be pre-set)."""
    sb, wpool, pp = pools["work"], pools["wt"], pools["psum"]
    aug = din < 128          # layers 1-3: ones row folded into xT / V / rhs2

    U_sb = wpool.tile([din, dout], F32, tag="U_sb")
    nc.sync.dma_start(out=U_sb[:], in_=U_d[:])
    vrows = din + 1 if aug else din
    V_sb = wpool.tile([vrows, dout], F32, tag="V_sb")
    nc.sync.dma_start(out=V_sb[:], in_=V_d[:])
    if not aug:
        c_sb = wpool.tile([1, dout], F32, tag="c_sb")
        nc.sync.dma_start(out=c_sb[:], in_=c_d[:])

    ones_col = consts["ones_col"]
    ones_row = consts["ones_row"]
    ident = consts["ident"]

    # squared norms -> -|x_j|^2, and the augmented moving tensor
    # rhs2 rows 0..din-1 = 2*x^T, row din = -|x_j|^2 (layers 1-3)
    sqx = sb.tile([din, N], F32, tag="sqx")
    nc.scalar.activation(sqx[:], xT[:din, :], AFT.Square)
    r2rows = din + 1 if aug else din
    rhs2 = sb.tile([r2rows, N], F32, tag="rhs2")
    nc.scalar.activation(rhs2[:din, :], xT[:din, :], AFT.Copy, scale=2.0)
    negsq_t = sb.tile([1, N], F32, tag="negsq")
    negsq = negsq_t[:]
    for c4 in range(4):
        cs = slice(c4 * 512, (c4 + 1) * 512)
        ps_sq = pp.tile([1, 512], F32, tag="ps_s")
        nc.tensor.matmul(out=ps_sq[:], lhsT=ones_col[:din, :], rhs=sqx[:, cs],
                         start=True, stop=True)
        nc.scalar.activation(negsq[:, cs], ps_sq[:], AFT.Copy, scale=-1.0)
    if aug:
        # engine writes must start at partition 0/32/64/96; DMA has no such
        # restriction, so place the -|x|^2 row at partition `din` via DMA
        nc.sync.dma_start(out=rhs2[din:din + 1, :], in_=negsq_t[:])

    # b' staging table first (only needs xT), so gathers can pipeline
    for r in range(NCH):
        rs = slice(r * P, (r + 1) * P)
        ps_b = pp.tile([P, dout], F32, tag="ps_ab")
        if aug:
            nc.tensor.matmul(out=ps_b[:], lhsT=xT[:, rs], rhs=V_sb[:],
                             start=True, stop=True)
        else:
            nc.tensor.matmul(out=ps_b[:], lhsT=xT[:, rs], rhs=V_sb[:],
                             start=True, stop=False)
            nc.tensor.matmul(out=ps_b[:], lhsT=ones_row[:, rs], rhs=c_sb[:],
                             start=False, stop=True)
        b_sb = sb.tile([P, dout], F32, tag="b_sb")
        nc.scalar.activation(b_sb[:], ps_b[:], AFT.Copy)
        nc.sync.dma_start(out=b_d[rs, :], in_=b_sb[:])

    idx = sb.tile([P, NCH * 24], U32, tag="idx")

    # per chunk: S + top-20 + gathers; combine-ops lag 2 chunks behind so the
    # DVE never stalls on the chunk's own gather DMAs.
    LAG = 0
    Gs = [None] * NCH

    def consume(r):
        rs = slice(r * P, (r + 1) * P)
        G = Gs[r]
        m = sb.tile([P, dout], F32, tag="m")
        nc.vector.tensor_reduce(
            out=m[:], in_=G[:].rearrange("p (k c) -> p c k", k=K, c=dout),
            axis=AXL.X, op=ALU.max)
        ps_a = pp.tile([P, dout], F32, tag="ps_ab")
        nc.tensor.matmul(out=ps_a[:], lhsT=xT[:din, rs], rhs=U_sb[:],
                         start=True, stop=True)
        xn = sb.tile([P, dout], F32, tag="xn")
        nc.vector.tensor_add(out=xn[:], in0=ps_a[:], in1=m[:])
        nc.vector.scalar_tensor_tensor(out=xn[:], in0=xn[:], scalar=0.2,
                                       in1=xn[:], op0=ALU.mult, op1=ALU.max)
        for dc in range((dout + P - 1) // P):
            w = min(P, dout - dc * P)
            ps_t = pp.tile([P, P], F32, tag="ps_t")
            nc.tensor.transpose(out=ps_t[:w, :], in_=xn[:, dc * P: dc * P + w],
                                identity=ident[:])
            nc.scalar.activation(out_tiles[dc][:w, rs], ps_t[:w, :], AFT.Copy)

    for r in range(NCH):
        rs = slice(r * P, (r + 1) * P)
        S = sb.tile([P, N], F32, tag="S")
        for c4 in range(4):
            cs = slice(c4 * 512, (c4 + 1) * 512)
            ps_s = pp.tile([P, 512], F32, tag="ps_s")
            if aug:
                nc.tensor.matmul(out=ps_s[:], lhsT=xT[:, rs], rhs=rhs2[:, cs],
                                 start=True, stop=True)
            else:
                nc.tensor.matmul(out=ps_s[:], lhsT=xT[:, rs], rhs=rhs2[:, cs],
                                 start=True, stop=False)
                nc.tensor.matmul(out=ps_s[:], lhsT=ones_row[:, rs],
                                 rhs=negsq[:, cs], start=False, stop=True)
            nc.scalar.activation(S[:, cs], ps_s[:], AFT.Copy)
        ib = r * 24
        for rnd in range(3):
            vals = sb.tile([P, 8], F32, tag="vals")
            nc.vector.max(out=vals[:], in_=S[:])
            nc.vector.max_index(out=idx[:, ib + 8 * rnd: ib + 8 * rnd + 8],
                                in_max=vals[:], in_values=S[:])
            if rnd < 2:
                nc.vector.match_replace(out=S[:], in_to_replace=vals[:],
                                        in_values=S[:], imm_value=NEG_BIG)

        # one indirect DMA per neighbor rank (HW: one descriptor/partition)
        G = pools["g"].tile([P, K * dout], F32, tag="G")
        Gs[r] = G
        for k in range(K):
            nc.gpsimd.indirect_dma_start(
                out=G[:, k * dout:(k + 1) * dout], out_offset=None, in_=b_d[:],
                in_offset=IndirectOffsetOnAxis(ap=idx[:, ib + k: ib + k + 1],
                                               axis=0))
        if r >= LAG:
            consume(r - LAG)
    for r in range(NCH - LAG, NCH):
        consume(r)


def _emit_head(nc, pools, consts, xts, dram):
    """xts: [x1T(64), x2T(64), x3T(128), x4aT(128), x4bT(128)] channel-major."""
    sb, wpool, pp = pools["work"], pools["wt"], pools["psum"]

    cm_sb = wpool.tile([P, 8], F32, tag="cm_sb")
    nc.sync.dma_start(out=cm_sb[:], in_=dram["cm"][:])
    ca_sb = wpool.tile([P, 4], F32, tag="ca_sb")
    nc.sync.dma_start(out=ca_sb[:], in_=dram["ca"][:])
    cb_sb = wpool.tile([P, 2], F32, tag="cb_sb")
    nc.sync.dma_start(out=cb_sb[:], in_=dram["cb"][:])
    bc_sb = wpool.tile([40, 1], F32, tag="bc_sb")
    nc.sync.dma_start(out=bc_sb[:], in_=dram["bc"][:])

    cchunks = [(0, 64, 0), (64, 128, 1), (128, 256, 2), (256, 384, 3),
               (384, 512, 4)]

    maxp = sb.tile([P, 8], F32, tag="maxp")
    sump = sb.tile([P, 8], F32, tag="sump")
    for mi in range(8):
        ms = slice(mi * P, (mi + 1) * P)
        mx4 = sb.tile([P, 4], F32, tag="mx4")
        sm4 = sb.tile([P, 4], F32, tag="sm4")
        for n4 in range(4):
            ns = slice(n4 * 512, (n4 + 1) * 512)
            ps_h = pp.tile([P, 512], F32, tag="ps_h")
            for i, (cs, ce, xi) in enumerate(cchunks):
                wm_sb = wpool.tile([P, P], F32, tag="wm_sb")
                nc.sync.dma_start(out=wm_sb[:ce - cs, :],
                                  in_=dram["Wm"][cs:ce, ms])
                nc.tensor.matmul(out=ps_h[:], lhsT=wm_sb[:ce - cs, :],
                                 rhs=xts[xi][: ce - cs, ns],
                                 start=(i == 0), stop=(i == 4))
            hm = sb.tile([P, 512], F32, tag="hm")
            nc.scalar.activation(hm[:], ps_h[:], AFT.Identity,
                                 bias=cm_sb[:, mi: mi + 1])
            nc.vector.scalar_tensor_tensor(out=hm[:], in0=hm[:], scalar=0.2,
                                           in1=hm[:], op0=ALU.mult, op1=ALU.max,
                                           accum_out=sm4[:, n4: n4 + 1])
            nc.vector.tensor_reduce(out=mx4[:, n4: n4 + 1], in_=hm[:],
                                    axis=AXL.X, op=ALU.max)
        nc.vector.tensor_reduce(out=maxp[:, mi: mi + 1], in_=mx4[:],
                                axis=AXL.X, op=ALU.max)
        nc.vector.tensor_reduce(out=sump[:, mi: mi + 1], in_=sm4[:],
                                axis=AXL.X, op=ALU.add)

    # pooled [2048] = [max(1024); sum(1024)/N(folded)] ;  ha = lrelu(@Wa + ca)
    ha = sb.tile([P, 4], F32, tag="ha")
    for m4 in range(4):
        ms = slice(m4 * P, (m4 + 1) * P)
        ps1 = pp.tile([P, 1], F32, tag="ps_h")
        for c in range(16):
            wa_sb = wpool.tile([P, P], F32, tag="wa_sb")
            nc.sync.dma_start(out=wa_sb[:],
                              in_=dram["Wa"][c * P:(c + 1) * P, ms])
            rhs = maxp[:, c: c + 1] if c < 8 else sump[:, c - 8: c - 7]
            nc.tensor.matmul(out=ps1[:], lhsT=wa_sb[:], rhs=rhs,
                             start=(c == 0), stop=(c == 15))
        nc.scalar.activation(ha[:, m4: m4 + 1], ps1[:], AFT.Identity,
                             bias=ca_sb[:, m4: m4 + 1])
        nc.vector.scalar_tensor_tensor(
            out=ha[:, m4: m4 + 1], in0=ha[:, m4: m4 + 1], scalar=0.2,
            in1=ha[:, m4: m4 + 1], op0=ALU.mult, op1=ALU.max)

    hb = sb.tile([P, 2], F32, tag="hb")
    for m2 in range(2):
        ms = slice(m2 * P, (m2 + 1) * P)
        ps1 = pp.tile([P, 1], F32, tag="ps_h")
        for c in range(4):
            wb_sb = wpool.tile([P, P], F32, tag="wb_sb")
            nc.sync.dma_start(out=wb_sb[:],
                              in_=dram["Wb"][c * P:(c + 1) * P, ms])
            nc.tensor.matmul(out=ps1[:], lhsT=wb_sb[:], rhs=ha[:, c: c + 1],
                             start=(c == 0), stop=(c == 3))
        nc.scalar.activation(hb[:, m2: m2 + 1], ps1[:], AFT.Identity,
                             bias=cb_sb[:, m2: m2 + 1])
        nc.vector.scalar_tensor_tensor(
            out=hb[:, m2: m2 + 1], in0=hb[:, m2: m2 + 1], scalar=0.2,
            in1=hb[:, m2: m2 + 1], op0=ALU.mult, op1=ALU.max)

    ps_o = pp.tile([40, 1], F32, tag="ps_h")
    for c in range(2):
        wc_sb = wpool.tile([P, 40], F32, tag="wc_sb")
        nc.sync.dma_start(out=wc_sb[:], in_=dram["Wc"][c * P:(c + 1) * P, :])
        nc.tensor.matmul(out=ps_o[:], lhsT=wc_sb[:], rhs=hb[:, c: c + 1],
                         start=(c == 0), stop=(c == 1))
    o_sb = sb.tile([40, 1], F32, tag="o_sb")
    nc.scalar.activation(o_sb[:], ps_o[:], AFT.Identity, bias=bc_sb[:])
    nc.sync.dma_start(out=dram["out"][:], in_=o_sb[:])


def build_program():
    nc = bacc.Bacc("TRN2", target_bir_lowering=False, debug=False,
                   enable_asserts=False)

    dram = {"pos": nc.dram_tensor("pos", [N, 3], F32, kind="ExternalInput")}
    for li, (din, dout) in enumerate(LAYERS, start=1):
        vrows = din + 1 if din < 128 else din
        dram[f"U{li}"] = nc.dram_tensor(f"U{li}", [din, dout], F32,
                                        kind="ExternalInput")
        dram[f"V{li}"] = nc.dram_tensor(f"V{li}", [vrows, dout], F32,
                                        kind="ExternalInput")
        if din == 128:
            dram[f"c{li}"] = nc.dram_tensor(f"c{li}", [1, dout], F32,
                                            kind="ExternalInput")
        dram[f"btab{li}"] = nc.dram_tensor(f"btab{li}", [N, dout], F32,
                                           kind="Internal")
    dram["Wm"] = nc.dram_tensor("Wm", [512, 1024], F32, kind="ExternalInput")
    dram["cm"] = nc.dram_tensor("cm", [P, 8], F32, kind="ExternalInput")
    dram["Wa"] = nc.dram_tensor("Wa", [2048, 512], F32, kind="ExternalInput")
    dram["ca"] = nc.dram_tensor("ca", [P, 4], F32, kind="ExternalInput")
    dram["Wb"] = nc.dram_tensor("Wb", [512, 256], F32, kind="ExternalInput")
    dram["cb"] = nc.dram_tensor("cb", [P, 2], F32, kind="ExternalInput")
    dram["Wc"] = nc.dram_tensor("Wc", [256, 40], F32, kind="ExternalInput")
    dram["bc"] = nc.dram_tensor("bc", [40, 1], F32, kind="ExternalInput")
    dram["out"] = nc.dram_tensor("out", [40, 1], F32, kind="ExternalOutput")

    with tile.TileContext(nc) as tc:
        from contextlib import ExitStack
        with ExitStack() as ctx:
            persist = ctx.enter_context(tc.tile_pool(name="persist", bufs=1))
            pools = {
                "work": ctx.enter_context(tc.tile_pool(name="work", bufs=2)),
                "g": ctx.enter_context(tc.tile_pool(name="g", bufs=2)),
                "wt": ctx.enter_context(tc.tile_pool(name="wt", bufs=3)),
            }

            ident = persist.tile([P, P], F32, tag="ident")
            make_identity(nc, ident[:])
            ones_col = persist.tile([P, 1], F32, tag="ones_col")
            nc.vector.memset(ones_col[:], 1.0)
            ones_row = persist.tile([1, N], F32, tag="ones_row")
            nc.vector.memset(ones_row[:], 1.0)
            consts = {"ident": ident, "ones_col": ones_col,
                      "ones_row": ones_row}

            # x feature tiles (channel-major), kept for the head concat
            x1 = persist.tile([65, N], F32, tag="x1")
            x2 = persist.tile([65, N], F32, tag="x2")
            x3 = persist.tile([P, N], F32, tag="x3")
            x4a = persist.tile([P, N], F32, tag="x4a")
            x4b = persist.tile([P, N], F32, tag="x4b")
            x0 = persist.tile([4, N], F32, tag="x0")

            nc.vector.memset(x0[:], 1.0)
            nc.sync.dma_start(out=x0[:3, :],
                              in_=dram["pos"][:].rearrange("n f -> f n"))
            nc.vector.memset(x1[64:65, :], 1.0)
            nc.vector.memset(x2[64:65, :], 1.0)

            outs = [[x1], [x2], [x3], [x4a, x4b]]
            xin = x0[:]
            with tc.tile_pool(name="psum_l", bufs=2, space="PSUM") as pp_l:
                pools["psum"] = pp_l
                for li, (din, dout) in enumerate(LAYERS, start=1):
                    _emit_edge_layer(nc, pools, consts, li, din, dout, xin,
                                     dram[f"U{li}"], dram[f"V{li}"],
                                     dram.get(f"c{li}"), dram[f"btab{li}"],
                                     outs[li - 1])
                    if li < 4:
                        xin = outs[li - 1][0][:]

            with tc.tile_pool(name="psum_h", bufs=2, space="PSUM") as pp_h:
                pools["psum"] = pp_h
                _emit_head(nc, pools, consts,
                           [x1[:], x2[:], x3[:], x4a[:], x4b[:]], dram)

    nc.compile()
    return nc


# --------------------------------------------------------------------------
# host-side weight folding
# --------------------------------------------------------------------------

def fold_inputs(inputs):
    d = {k: np.ascontiguousarray(np.asarray(v, dtype=np.float32))
         for k, v in inputs.items()}

    def bn(name):
        s = d["g" + name] / np.sqrt(d["var" + name] + np.float32(EPS))
        c = d["beta" + name] - d["mu" + name] * s
        return s.astype(np.float32), c.astype(np.float32)

    common = {}
    for li, (din, dout) in enumerate(LAYERS, start=1):
        W = d[f"W{li}"]
        s, c = bn(str(li))
        Ws = (W * s[None, :]).astype(np.float32)
        common[f"U{li}"] = np.ascontiguousarray(Ws[:din] - Ws[din:])
        V = Ws[din:]
        if din < 128:
            V = np.concatenate([V, c[None, :]], axis=0)
            common[f"V{li}"] = np.ascontiguousarray(V)
        else:
            common[f"V{li}"] = np.ascontiguousarray(V)
            common[f"c{li}"] = np.ascontiguousarray(c[None, :])

    s, c = bn("m")
    common["Wm"] = np.ascontiguousarray(d["Wm"] * s[None, :])
    common["cm"] = np.ascontiguousarray(c.reshape(8, P).T)

    s, c = bn("a")
    Wa = (d["Wa"] * s[None, :]).astype(np.float32)
    Wa[1024:] /= np.float32(N)
    common["Wa"] = np.ascontiguousarray(Wa)
    common["ca"] = np.ascontiguousarray(c.reshape(4, P).T)

    s, _ = bn("b")
    common["Wb"] = np.ascontiguousarray(d["Wb"] * s[None, :])
    cb = ((d["bias_b"] - d["mub"]) * s + d["betab"]).astype(np.float32)
    common["cb"] = np.ascontiguousarray(cb.reshape(2, P).T)

    common["Wc"] = np.ascontiguousarray(d["Wc"])
    common["bc"] = np.ascontiguousarray(d["bias_c"].reshape(40, 1))

    in_maps = [{**common, "pos": np.ascontiguousarray(d["pos"][g])}
               for g in range(B)]
    return in_maps


_CACHE = {}


def kernel(**inputs):
    if "nc" not in _CACHE:
        _CACHE["nc"] = build_program()
    nc = _CACHE["nc"]
    in_maps = fold_inputs(inputs)
    res = run_bass_kernel_spmd(nc, in_maps, core_ids=list(range(B)))
    out = np.stack([np.asarray(r["out"]).reshape(40) for r in res.results])
    return out.astype(np.float32)


# revision 16
# speedup vs baseline: 1.1497x; 1.0273x over previous
"""DGCNN Bass/Trainium2 kernel.

Strategy (data-parallel over the 8 graphs, one graph per NeuronCore):

Per EdgeConv layer, exploit:
  [x_i, x_j - x_i] @ W  ==  x_i @ (W_hi - W_lo) + x_j @ W_lo
and the monotonicity of LeakyReLU + per-channel affine BN (folded into the
weights on the host) so that
  max_k LReLU(BN(e_ijk))  ==  LReLU(a'_i + max_k b'_{j_k})
with a' = x@U' (+bias via c), b' = x@V' + c.

Neighbor selection maximizes S[i,j] = 2 x_i.x_j - |x_j|^2  (equivalent
ordering to minimizing the squared distance).  S row chunks are computed on
the TensorEngine, top-20 per row extracted with the DVE max8 /
match_replace / max_index instructions (3 rounds of 8), and the b' rows of
the selected neighbors are fetched with an indirect (gather) DMA from a
DRAM staging table, then max-reduced on the DVE.

Everything is fp32.  All feature maps stay on-chip in channel-major
[F, N] layout; per-chunk point-major results are transposed back with the
TensorEngine.
"""

import numpy as np

import concourse.bass as bass
import concourse.bacc as bacc
import concourse.mybir as mybir
import concourse.tile as tile
from concourse.bass import IndirectOffsetOnAxis
from concourse.bass_utils import run_bass_kernel_spmd
from concourse.masks import make_identity

F32 = mybir.dt.float32
U32 = mybir.dt.uint32
AFT = mybir.ActivationFunctionType
ALU = mybir.AluOpType
AXL = mybir.AxisListType

N = 2048
P = 128
NCH = N // P            # 16 row chunks
K = 20
NEG_BIG = -1.0e38
EPS = 1e-5
B = 8
LAYERS = [(3, 64), (64, 64), (64, 128), (128, 256)]


# --------------------------------------------------------------------------
# program builder (single core; one full graph per core)
# --------------------------------------------------------------------------

def _emit_edge_layer(nc, pools, consts, li, din, dout, xT, U_d, V_d, c_d, b_d,
                     out_tiles):
    """xT: AP [din(+1), N] channel-major input; for layers 1-3 the extra last
    row holds ones (contract-augmentation).  out_tiles: SBUF tiles receiving
    the channel-major output (rows beyond dout, if any, must be pre-set)."""
    sb, wpool, pp = pools["work"], pools["wt"], pools["psum"]
    aug = din < 128          # layers 1-3: ones row folded into xT / V / rhs2

    U_sb = wpool.tile([din, dout], F32, tag="U_sb")
    nc.sync.dma_start(out=U_sb[:], in_=U_d[:])
    vrows = din + 1 if aug else din
    V_sb = wpool.tile([vrows, dout], F32, tag="V_sb")
    nc.sync.dma_start(out=V_sb[:], in_=V_d[:])
    if not aug:
        c_sb = wpool.tile([1, dout], F32, tag="c_sb")
        nc.sync.dma_start(out=c_sb[:], in_=c_d[:])

    ones_col = consts["ones_col"]
    ones_row = consts["ones_row"]
    ident = consts["ident"]

    # squared norms -> -|x_j|^2, and the augmented moving tensor
    # rhs2 rows 0..din-1 = 2*x^T, row din = -|x_j|^2 (layers 1-3)
    sqx = sb.tile([din, N], F32, tag="sqx")
    nc.scalar.activation(sqx[:], xT[:din, :], AFT.Square)
    r2rows = din + 1 if aug else din
    rhs2 = sb.tile([r2rows, N], F32, tag="rhs2")
    nc.scalar.activation(rhs2[:din, :], xT[:din, :], AFT.Copy, scale=2.0)
    negsq_t = sb.tile([1, N], F32, tag="negsq")
    negsq = negsq_t[:]
    for c4 in range(4):
        cs = slice(c4 * 512, (c4 + 1) * 512)
        ps_sq = pp.tile([1, 512], F32, tag="ps_s")
        nc.tensor.matmul(out=ps_sq[:], lhsT=ones_col[:din, :], rhs=sqx[:, cs],
                         start=True, stop=True)
        nc.scalar.activation(negsq[:, cs], ps_sq[:], AFT.Copy, scale=-1.0)
    if aug:
        # engine writes must start at partition 0/32/64/96; DMA has no such
        # restriction, so place the -|x|^2 row at partition `din` via DMA
        nc.sync.dma_start(out=rhs2[din:din + 1, :], in_=negsq_t[:])

    # b' staging table first (only needs xT), so gathers can pipeline
    for r in range(NCH):
        rs = slice(r * P, (r + 1) * P)
        ps_b = pp.tile([P, dout], F32, tag="ps_ab")
        if aug:
            nc.tensor.matmul(out=ps_b[:], lhsT=xT[:, rs], rhs=V_sb[:],
                             start=True, stop=True)
        else:
            nc.tensor.matmul(out=ps_b[:], lhsT=xT[:, rs], rhs=V_sb[:],
                             start=True, stop=False)
            nc.tensor.matmul(out=ps_b[:], lhsT=ones_row[:, rs], rhs=c_sb[:],
                             start=False, stop=True)
        b_sb = sb.tile([P, dout], F32, tag="b_sb")
        nc.scalar.activation(b_sb[:], ps_b[:], AFT.Copy)
        nc.sync.dma_start(out=b_d[rs, :], in_=b_sb[:])

    idx = sb.tile([P, NCH * 24], U32, tag="idx")

    # per chunk: S + top-20 + gathers; combine-ops lag 2 chunks behind so the
    # DVE never stalls on the chunk's own gather DMAs.
    LAG = 0
    Gs = [None] * NCH

    def consume(r):
        rs = slice(r * P, (r + 1) * P)
        G = Gs[r]
        m = sb.tile([P, dout], F32, tag="m")
        nc.vector.tensor_reduce(
            out=m[:], in_=G[:].rearrange("p (k c) -> p c k", k=K, c=dout),
            axis=AXL.X, op=ALU.max)
        ps_a = pp.tile([P, dout], F32, tag="ps_ab")
        nc.tensor.matmul(out=ps_a[:], lhsT=xT[:din, rs], rhs=U_sb[:],
                         start=True, stop=True)
        xn = sb.tile([P, dout], F32, tag="xn")
        nc.vector.tensor_add(out=xn[:], in0=ps_a[:], in1=m[:])
        nc.vector.scalar_tensor_tensor(out=xn[:], in0=xn[:], scalar=0.2,
                                       in1=xn[:], op0=ALU.mult, op1=ALU.max)
        for dc in range((dout + P - 1) // P):
            w = min(P, dout - dc * P)
            ps_t = pp.tile([P, P], F32, tag="ps_t")
            nc.tensor.transpose(out=ps_t[:w, :], in_=xn[:, dc * P: dc * P + w],
                                identity=ident[:])
            nc.scalar.activation(out_tiles[dc][:w, rs], ps_t[:w, :], AFT.Copy)

    for r in range(NCH):
        rs = slice(r * P, (r + 1) * P)
        S = sb.tile([P, N], F32, tag="S")
        for c4 in range(4):
            cs = slice(c4 * 512, (c4 + 1) * 512)
            ps_s = pp.tile([P, 512], F32, tag="ps_s")
            if aug:
                nc.tensor.matmul(out=ps_s[:], lhsT=xT[:, rs], rhs=rhs2[:, cs],
                                 start=True, stop=True)
            else:
                nc.tensor.matmul(out=ps_s[:], lhsT=xT[:, rs], rhs=rhs2[:, cs],
                                 start=True, stop=False)
                nc.tensor.matmul(out=ps_s[:], lhsT=ones_row[:, rs],
                                 rhs=negsq[:, cs], start=False, stop=True)
            nc.scalar.activation(S[:, cs], ps_s[:], AFT.Copy)
        ib = r * 24
        for rnd in range(3):
            vals = sb.tile([P, 8], F32, tag="vals")
            nc.vector.max(out=vals[:], in_=S[:])
            nc.vector.max_index(out=idx[:, ib + 8 * rnd: ib + 8 * rnd + 8],
                                in_max=vals[:], in_values=S[:])
            if rnd < 2:
                nc.vector.match_replace(out=S[:], in_to_replace=vals[:],
                                        in_values=S[:], imm_value=NEG_BIG)

        # one indirect DMA per neighbor rank (HW: one descriptor/partition)
        G = pools["g"].tile([P, K * dout], F32, tag="G")
        Gs[r] = G
        for k in range(K):
            nc.gpsimd.indirect_dma_start(
                out=G[:, k * dout:(k + 1) * dout], out_offset=None, in_=b_d[:],
                in_offset=IndirectOffsetOnAxis(ap=idx[:, ib + k: ib + k + 1],
                                               axis=0))
        if r >= LAG:
            consume(r - LAG)
    for r in range(NCH - LAG, NCH):
        consume(r)


def _emit_head(nc, pools, consts, xts, dram):
    """xts: [x1T(64), x2T(64), x3T(128), x4aT(128), x4bT(128)] channel-major."""
    sb, wpool, pp = pools["work"], pools["wt"], pools["psum"]

    cm_sb = wpool.tile([P, 8], F32, tag="cm_sb")
    nc.sync.dma_start(out=cm_sb[:], in_=dram["cm"][:])
    ca_sb = wpool.tile([P, 4], F32, tag="ca_sb")
    nc.sync.dma_start(out=ca_sb[:], in_=dram["ca"][:])
    cb_sb = wpool.tile([P, 2], F32, tag="cb_sb")
    nc.sync.dma_start(out=cb_sb[:], in_=dram["cb"][:])
    bc_sb = wpool.tile([40, 1], F32, tag="bc_sb")
    nc.sync.dma_start(out=bc_sb[:], in_=dram["bc"][:])

    cchunks = [(0, 64, 0), (64, 128, 1), (128, 256, 2), (256, 384, 3),
               (384, 512, 4)]

    maxp = sb.tile([P, 8], F32, tag="maxp")
    sump = sb.tile([P, 8], F32, tag="sump")
    for mi in range(8):
        ms = slice(mi * P, (mi + 1) * P)
        mx4 = sb.tile([P, 4], F32, tag="mx4")
        sm4 = sb.tile([P, 4], F32, tag="sm4")
        wm_big = wpool.tile([P, 512], F32, tag="wm_big")
        nc.sync.dma_start(
            out=wm_big[:].rearrange("p (b c) -> p b c", b=4, c=P),
            in_=dram["Wm"][:, ms].rearrange("(b p) c -> p b c", p=P, b=4))
        wm_x2 = wpool.tile([64, P], F32, tag="wm_x2")
        nc.sync.dma_start(out=wm_x2[:], in_=dram["Wm"][64:128, ms])
        lhs_slices = [wm_big[0:64, 0:P], wm_x2[:],
                      wm_big[:, P:2 * P], wm_big[:, 2 * P:3 * P],
                      wm_big[:, 3 * P:4 * P]]
        for n4 in range(4):
            ns = slice(n4 * 512, (n4 + 1) * 512)
            ps_h = pp.tile([P, 512], F32, tag="ps_h")
            for i, (cs, ce, xi) in enumerate(cchunks):
                nc.tensor.matmul(out=ps_h[:], lhsT=lhs_slices[i],
                                 rhs=xts[xi][: ce - cs, ns],
                                 start=(i == 0), stop=(i == 4))
            hm = sb.tile([P, 512], F32, tag="hm")
            nc.scalar.activation(hm[:], ps_h[:], AFT.Identity,
                                 bias=cm_sb[:, mi: mi + 1])
            nc.vector.scalar_tensor_tensor(out=hm[:], in0=hm[:], scalar=0.2,
                                           in1=hm[:], op0=ALU.mult, op1=ALU.max,
                                           accum_out=sm4[:, n4: n4 + 1])
            nc.vector.tensor_reduce(out=mx4[:, n4: n4 + 1], in_=hm[:],
                                    axis=AXL.X, op=ALU.max)
        nc.vector.tensor_reduce(out=maxp[:, mi: mi + 1], in_=mx4[:],
                                axis=AXL.X, op=ALU.max)
        nc.vector.tensor_reduce(out=sump[:, mi: mi + 1], in_=sm4[:],
                                axis=AXL.X, op=ALU.add)

    # pooled [2048] = [max(1024); sum(1024)/N(folded)] ;  ha = lrelu(@Wa + ca)
    ha = sb.tile([P, 4], F32, tag="ha")
    for m4 in range(4):
        ms = slice(m4 * P, (m4 + 1) * P)
        ps1 = pp.tile([P, 1], F32, tag="ps_h")
        wa_bigs = []
        for h in range(2):
            wa_big = wpool.tile([P, 8 * P], F32, tag="wa_big")
            nc.sync.dma_start(
                out=wa_big[:].rearrange("p (b c) -> p b c", b=8, c=P),
                in_=dram["Wa"][h * 1024:(h + 1) * 1024, ms].rearrange(
                    "(b p) c -> p b c", p=P, b=8))
            wa_bigs.append(wa_big)
        for c in range(16):
            rhs = maxp[:, c: c + 1] if c < 8 else sump[:, c - 8: c - 7]
            nc.tensor.matmul(out=ps1[:], lhsT=wa_bigs[c // 8][:, (c % 8) * P:
                                                              (c % 8 + 1) * P],
                             rhs=rhs, start=(c == 0), stop=(c == 15))
        nc.scalar.activation(ha[:, m4: m4 + 1], ps1[:], AFT.Identity,
                             bias=ca_sb[:, m4: m4 + 1])
        nc.vector.scalar_tensor_tensor(
            out=ha[:, m4: m4 + 1], in0=ha[:, m4: m4 + 1], scalar=0.2,
            in1=ha[:, m4: m4 + 1], op0=ALU.mult, op1=ALU.max)

    hb = sb.tile([P, 2], F32, tag="hb")
    for m2 in range(2):
        ms = slice(m2 * P, (m2 + 1) * P)
        ps1 = pp.tile([P, 1], F32, tag="ps_h")
        for c in range(4):
            wb_sb = wpool.tile([P, P], F32, tag="wb_sb")
            nc.sync.dma_start(out=wb_sb[:],
                              in_=dram["Wb"][c * P:(c + 1) * P, ms])
            nc.tensor.matmul(out=ps1[:], lhsT=wb_sb[:], rhs=ha[:, c: c + 1],
                             start=(c == 0), stop=(c == 3))
        nc.scalar.activation(hb[:, m2: m2 + 1], ps1[:], AFT.Identity,
                             bias=cb_sb[:, m2: m2 + 1])
        nc.vector.scalar_tensor_tensor(
            out=hb[:, m2: m2 + 1], in0=hb[:, m2: m2 + 1], scalar=0.2,
            in1=hb[:, m2: m2 + 1], op0=ALU.mult, op1=ALU.max)

    ps_o = pp.tile([40, 1], F32, tag="ps_h")
    for c in range(2):
        wc_sb = wpool.tile([P, 40], F32, tag="wc_sb")
        nc.sync.dma_start(out=wc_sb[:], in_=dram["Wc"][c * P:(c + 1) * P, :])
        nc.tensor.matmul(out=ps_o[:], lhsT=wc_sb[:], rhs=hb[:, c: c + 1],
                         start=(c == 0), stop=(c == 1))
    o_sb = sb.tile([40, 1], F32, tag="o_sb")
    nc.scalar.activation(o_sb[:], ps_o[:], AFT.Identity, bias=bc_sb[:])
    nc.sync.dma_start(out=dram["out"][:], in_=o_sb[:])


def build_program():
    nc = bacc.Bacc("TRN2", target_bir_lowering=False, debug=False,
                   enable_asserts=False)

    dram = {"pos": nc.dram_tensor("pos", [N, 3], F32, kind="ExternalInput")}
    for li, (din, dout) in enumerate(LAYERS, start=1):
        vrows = din + 1 if din < 128 else din
        dram[f"U{li}"] = nc.dram_tensor(f"U{li}", [din, dout], F32,
                                        kind="ExternalInput")
        dram[f"V{li}"] = nc.dram_tensor(f"V{li}", [vrows, dout], F32,
                                        kind="ExternalInput")
        if din == 128:
            dram[f"c{li}"] = nc.dram_tensor(f"c{li}", [1, dout], F32,
                                            kind="ExternalInput")
        dram[f"btab{li}"] = nc.dram_tensor(f"btab{li}", [N, dout], F32,
                                           kind="Internal")
    dram["Wm"] = nc.dram_tensor("Wm", [512, 1024], F32, kind="ExternalInput")
    dram["cm"] = nc.dram_tensor("cm", [P, 8], F32, kind="ExternalInput")
    dram["Wa"] = nc.dram_tensor("Wa", [2048, 512], F32, kind="ExternalInput")
    dram["ca"] = nc.dram_tensor("ca", [P, 4], F32, kind="ExternalInput")
    dram["Wb"] = nc.dram_tensor("Wb", [512, 256], F32, kind="ExternalInput")
    dram["cb"] = nc.dram_tensor("cb", [P, 2], F32, kind="ExternalInput")
    dram["Wc"] = nc.dram_tensor("Wc", [256, 40], F32, kind="ExternalInput")
    dram["bc"] = nc.dram_tensor("bc", [40, 1], F32, kind="ExternalInput")
    dram["out"] = nc.dram_tensor("out", [40, 1], F32, kind="ExternalOutput")

    with tile.TileContext(nc) as tc:
        from contextlib import ExitStack
        with ExitStack() as ctx:
            persist = ctx.enter_context(tc.tile_pool(name="persist", bufs=1))
            pools = {
                "work": ctx.enter_context(tc.tile_pool(name="work", bufs=2)),
                "g": ctx.enter_context(tc.tile_pool(name="g", bufs=2)),
                "wt": ctx.enter_context(tc.tile_pool(name="wt", bufs=3)),
            }

            ident = persist.tile([P, P], F32, tag="ident")
            make_identity(nc, ident[:])
            ones_col = persist.tile([P, 1], F32, tag="ones_col")
            nc.vector.memset(ones_col[:], 1.0)
            ones_row = persist.tile([1, N], F32, tag="ones_row")
            nc.vector.memset(ones_row[:], 1.0)
            consts = {"ident": ident, "ones_col": ones_col,
                      "ones_row": ones_row}

            # x feature tiles (channel-major), kept for the head concat
            x1 = persist.tile([65, N], F32, tag="x1")
            x2 = persist.tile([65, N], F32, tag="x2")
            x3 = persist.tile([P, N], F32, tag="x3")
            x4a = persist.tile([P, N], F32, tag="x4a")
            x4b = persist.tile([P, N], F32, tag="x4b")
            x0 = persist.tile([4, N], F32, tag="x0")

            nc.vector.memset(x0[:], 1.0)
            nc.sync.dma_start(out=x0[:3, :],
                              in_=dram["pos"][:].rearrange("n f -> f n"))
            nc.vector.memset(x1[64:65, :], 1.0)
            nc.vector.memset(x2[64:65, :], 1.0)

            outs = [[x1], [x2], [x3], [x4a, x4b]]
            xin = x0[:]
            with tc.tile_pool(name="psum_l", bufs=2, space="PSUM") as pp_l:
                pools["psum"] = pp_l
                for li, (din, dout) in enumerate(LAYERS, start=1):
                    _emit_edge_layer(nc, pools, consts, li, din, dout, xin,
                                     dram[f"U{li}"], dram[f"V{li}"],
                                     dram.get(f"c{li}"), dram[f"btab{li}"],
                                     outs[li - 1])
                    if li < 4:
                        xin = outs[li - 1][0][:]

            with tc.tile_pool(name="psum_h", bufs=2, space="PSUM") as pp_h:
                pools["psum"] = pp_h
                _emit_head(nc, pools, consts,
                           [x1[:], x2[:], x3[:], x4a[:], x4b[:]], dram)

    nc.compile()
    return nc


# --------------------------------------------------------------------------
# host-side weight folding
# --------------------------------------------------------------------------

def fold_inputs(inputs):
    d = {k: np.ascontiguousarray(np.asarray(v, dtype=np.float32))
         for k, v in inputs.items()}

    def bn(name):
        s = d["g" + name] / np.sqrt(d["var" + name] + np.float32(EPS))
        c = d["beta" + name] - d["mu" + name] * s
        return s.astype(np.float32), c.astype(np.float32)

    common = {}
    for li, (din, dout) in enumerate(LAYERS, start=1):
        W = d[f"W{li}"]
        s, c = bn(str(li))
        Ws = (W * s[None, :]).astype(np.float32)
        common[f"U{li}"] = np.ascontiguousarray(Ws[:din] - Ws[din:])
        V = Ws[din:]
        if din < 128:
            V = np.concatenate([V, c[None, :]], axis=0)
            common[f"V{li}"] = np.ascontiguousarray(V)
        else:
            common[f"V{li}"] = np.ascontiguousarray(V)
            common[f"c{li}"] = np.ascontiguousarray(c[None, :])

    s, c = bn("m")
    common["Wm"] = np.ascontiguousarray(d["Wm"] * s[None, :])
    common["cm"] = np.ascontiguousarray(c.reshape(8, P).T)

    s, c = bn("a")
    Wa = (d["Wa"] * s[None, :]).astype(np.float32)
    Wa[1024:] /= np.float32(N)
    common["Wa"] = np.ascontiguousarray(Wa)
    common["ca"] = np.ascontiguousarray(c.reshape(4, P).T)

    s, _ = bn("b")
    common["Wb"] = np.ascontiguousarray(d["Wb"] * s[None, :])
    cb = ((d["bias_b"] - d["mub"]) * s + d["betab"]).astype(np.float32)
    common["cb"] = np.ascontiguousarray(cb.reshape(2, P).T)

    common["Wc"] = np.ascontiguousarray(d["Wc"])
    common["bc"] = np.ascontiguousarray(d["bias_c"].reshape(40, 1))

    in_maps = [{**common, "pos": np.ascontiguousarray(d["pos"][g])}
               for g in range(B)]
    return in_maps


_CACHE = {}


def kernel(**inputs):
    if "nc" not in _CACHE:
        _CACHE["nc"] = build_program()
    nc = _CACHE["nc"]
    in_maps = fold_inputs(inputs)
    res = run_bass_kernel_spmd(nc, in_maps, core_ids=list(range(B)))
    out = np.stack([np.asarray(r["out"]).reshape(40) for r in res.results])
    return out.astype(np.float32)
